# revision 1
# baseline (speedup 1.0000x reference)
"""ConvCapsules2d Trainium2 kernel (Bass/Tile), SPMD over 8 NeuronCores.

Full problem:
  poses (16,32,16,14,14) f32, W (32,32,16,3,3) f32
  V[n,b,c,d,f,g,k,l] = W[b,c,d,k,l] * sum_p poses[n,b,p,2f+k,2g+l]
  V: (16,32,32,16,6,6,3,3) f32  (~340 MB -> memory/write bound)

Sharding: data-parallel over batch N: core i computes n in [2i, 2i+2).

Per-core layout: SBUF partition q = n*64 + b*2 + clo (n in 0..1, b in 0..31,
clo in 0..1), with output channel c = 2*m + clo split into 16 c-pairs m.
Free dims carry (d, f, g, k, l) = 5184 contiguous elements. Each of the 32
output stores is a (64 partition x 5184) DMA whose DRAM access pattern is
[[165888,32],[5184,2],[1,5184]] — outer count 32 so the descriptor spray
engages the full SDMA engine set (the previous (n,clo,b) ordering balanced
to an outer count of 2, which ran at ~55 GB/s instead of ~400 GB/s).

W is pre-permuted on the host to (64, 2304) = (b*2+clo, m*144+d*9+kl) so the
whole weight load is two contiguous full-spray DMAs. The n-duplication of
poses across the two 64-partition halves comes from re-reading the small
poses input; no cross-partition traffic on chip.
"""
import numpy as np

import concourse.bacc as bacc
import concourse.mybir as mybir
from concourse.tile import TileContext
from concourse import bass_utils

# ---- problem constants (hardcoded per contest contract) ----
NTOT, B, P, H = 16, 32, 16, 14
C, D, K, S = 32, 16, 3, 2
F = (H - K) // S + 1          # 6
FF, KK = F * F, K * K         # 36, 9
NCORES = 8
N = NTOT // NCORES            # 2 batches per core
NPART = 128
M = C // 2                    # 16 c-pairs
UNIT = D * FF * KK            # 5184 elements per (partition, m)


def _build(nc):
    # Both inputs arrive host-pre-arranged as one (128, free) row per SBUF
    # partition (q = n*64 + b*2 + clo), so each load is a single contiguous
    # full-spray DMA.
    poses = nc.dram_tensor("poses", (NPART, P * H * H), mybir.dt.float32, kind="ExternalInput")
    Wt = nc.dram_tensor("W", (NPART, M * D * KK), mybir.dt.float32, kind="ExternalInput")
    V = nc.dram_tensor("V", (N, B, C, D, F, F, K, K), mybir.dt.float32, kind="ExternalOutput")

    with TileContext(nc) as tc:
        with tc.tile_pool(name="const", bufs=1) as const_pool, \
             tc.tile_pool(name="work", bufs=2) as wpool, \
             tc.tile_pool(name="out", bufs=4) as opool:
            # ---- loads; partition q = n*64 + b*2 + clo
            poses_sb = const_pool.tile([NPART, P * H * H], mybir.dt.float32)
            nc.sync.dma_start(out=poses_sb[:], in_=poses.ap())

            W_sb = const_pool.tile([NPART, M * D * KK], mybir.dt.float32)
            nc.sync.dma_start(out=W_sb[:], in_=Wt.ap())

            # ---- s-phase: sum over P (binary tree of wide adds), then unfold
            HH = H * H
            acc = wpool.tile([NPART, HH], mybir.dt.float32, tag="acc")
            tmp = wpool.tile([NPART, HH * 8], mybir.dt.float32, tag="tmp")
            nc.vector.tensor_add(out=tmp[:, :HH * 8],
                                 in0=poses_sb[:, :HH * 8],
                                 in1=poses_sb[:, HH * 8:])
            nc.vector.tensor_add(out=tmp[:, :HH * 4],
                                 in0=tmp[:, :HH * 4],
                                 in1=tmp[:, HH * 4:HH * 8])
            nc.vector.tensor_add(out=tmp[:, :HH * 2],
                                 in0=tmp[:, :HH * 2],
                                 in1=tmp[:, HH * 2:HH * 4])
            nc.vector.tensor_add(out=acc[:], in0=tmp[:, :HH], in1=tmp[:, HH:HH * 2])

            s2 = wpool.tile([NPART, FF * KK], mybir.dt.float32, tag="s2")
            acc_v = acc[:].rearrange("q (i j) -> q i j", i=H)
            s2_v = s2[:].rearrange("q (f g k l) -> q f g k l", f=F, g=F, k=K)
            for k in range(K):
                for l in range(K):
                    nc.vector.tensor_copy(out=s2_v[:, :, :, k, l],
                                          in_=acc_v[:, k:k + 2 * F - 1:2, l:l + 2 * F - 1:2])

            # ---- multiply + store per c-pair m
            vap = V.ap().rearrange("n b (m clo) d f g k l -> n m b clo (d f g k l)", clo=2)
            w_all = W_sb[:].rearrange("q (m d kl) -> q m d kl", m=M, d=D)
            s_bc = s2[:].rearrange("q (fg kl) -> q fg kl", kl=KK)[:, None, :, :] \
                        .broadcast_to((NPART, D, FF, KK))
            for m in range(M):
                out_t = opool.tile([NPART, UNIT], mybir.dt.float32, tag="out")
                out_v = out_t[:].rearrange("q (d fg kl) -> q d fg kl", d=D, fg=FF)
                w_view = w_all[:, m, :, None, :].broadcast_to((NPART, D, FF, KK))
                nc.vector.tensor_mul(out=out_v, in0=w_view, in1=s_bc)
                for n in range(N):
                    nc.sync.dma_start(out=vap[n, m], in_=out_t[n * 64:(n + 1) * 64, :])
    return nc


def permute_W(W: np.ndarray) -> np.ndarray:
    """(B, C, D, K, K) -> (128, M*D*KK): row n*64+b*2+clo holds W[b, 2m+clo, d, k, l]."""
    Wp = W.reshape(B, M, 2, D, KK).transpose(0, 2, 1, 3, 4).reshape(2 * B, M * D * KK)
    return np.ascontiguousarray(np.concatenate([Wp, Wp], axis=0))


def dup_poses(poses_shard: np.ndarray) -> np.ndarray:
    """(N, B, P, H, H) core shard -> (128, P*H*H): row n*64+b*2+clo = poses[n, b]."""
    flat = poses_shard.reshape(N, B, 1, P * H * H)
    return np.ascontiguousarray(np.broadcast_to(flat, (N, B, 2, P * H * H))
                                .reshape(NPART, P * H * H))


_cached_nc = None


def _get_nc():
    global _cached_nc
    if _cached_nc is None:
        nc = bacc.Bacc("TRN2", target_bir_lowering=False)
        _build(nc)
        nc.compile()
        _cached_nc = nc
    return _cached_nc


def run_spmd(poses: np.ndarray, W: np.ndarray, **spmd_kwargs):
    """Shard, run on 8 cores, gather. Returns (V_full, BassKernelResults)."""
    poses = np.ascontiguousarray(np.asarray(poses, dtype=np.float32))
    W = np.ascontiguousarray(np.asarray(W, dtype=np.float32))
    assert poses.shape == (NTOT, B, P, H, H), poses.shape
    assert W.shape == (B, C, D, K, K), W.shape
    Wp = permute_W(W)
    nc = _get_nc()
    in_maps = [{"poses": dup_poses(poses[i * N:(i + 1) * N]), "W": Wp}
               for i in range(NCORES)]
    res = bass_utils.run_bass_kernel_spmd(nc, in_maps, core_ids=list(range(NCORES)),
                                          **spmd_kwargs)
    V = np.concatenate([r["V"] for r in res.results], axis=0)
    return V, res


def kernel(poses: np.ndarray, W: np.ndarray) -> np.ndarray:
    import time
    last_err = None
    for attempt in range(3):
        try:
            V, _ = run_spmd(poses, W)
            return V
        except Exception as e:  # transient NRT/axon device errors: poke + retry
            last_err = e
            time.sleep(2.0)
            try:
                import jax, jax.numpy as jnp
                jnp.sum(jnp.ones((8, 8))).block_until_ready()
            except Exception:
                pass
    raise last_err



# revision 4
# speedup vs baseline: 2.3632x; 2.3632x over previous
"""ConvCapsules2d Trainium2 kernel (Bass/Tile), SPMD over 8 NeuronCores.

Full problem:
  poses (16,32,16,14,14) f32, W (32,32,16,3,3) f32
  V[n,b,c,d,f,g,k,l] = W[b,c,d,k,l] * sum_p poses[n,b,p,2f+k,2g+l]
  V: (16,32,32,16,6,6,3,3) f32  (~340 MB -> memory/write bound)

Sharding: data-parallel over batch N: core i computes n in [2i, 2i+2).

The per-core job is pure HBM-write roofline (per-core DMA peak ~358 GB/s).
In f32 the 42.5 MB/core output store floor is ~119 us; the only big lever is
precision: V is computed and stored as f16 (rel err ~1e-3, tolerance 2e-2)
and upcast to f32 on the host during the gather. Inputs are f16 too and are
read exactly once with no duplication:

  partition q = b*4 + c4 (c4 = c div 8); row q handles output channels
  c in [8*c4, 8*c4+8) for both batches n.
  - poses row q holds poses[n, b, p, :, :] for p in [4*c4, 4*c4+4) -- the
    P-sum is split across the 4 partitions of a b-group. Two in-row tree
    adds reduce p4; the cross-partition group-of-4 reduce (replicated back
    to all 4 rows) is one PE matmul with a block-diagonal 0/1 matrix
    (kron(I32, ones(4,4))) into PSUM.
  - W row q = W[b, 8*c4:8*c4+8] is W.reshape(128, 1152) -- no permute.
  - 9 strided copies unfold PSUM s(14,14) -> s(f,g,k,l) per n, cast f16.
  - 4 muls (n x channel-quad) of 20736 f16 elems/partition feed 4 output
    stores whose DRAM pattern is [[b:32],[c4:4],[contig 41.5KB]] -- 128
    descriptors/store, full 16-engine spray.
Input loads ride the Activation HWDGE queue so the next rep's loads overlap
this rep's stores on the SP queue.
"""
import numpy as np

import concourse.bacc as bacc
import concourse.mybir as mybir
from concourse.tile import TileContext
from concourse import bass_utils

# ---- problem constants (hardcoded per contest contract) ----
NTOT, B, P, H = 16, 32, 16, 14
C, D, K, S = 32, 16, 3, 2
F = (H - K) // S + 1          # 6
FF, KK = F * F, K * K         # 36, 9
HH = H * H                    # 196
NCORES = 8
N = NTOT // NCORES            # 2 batches per core
NPART = 128
XPT = 4                       # output channels per store tile
TPN = 8 // XPT                # store tiles per n per row
UNIT = D * FF * KK            # 5184 elements per (n, b, c) block
TILE_FREE = XPT * UNIT        # 20736


def _v_ap(ap):
    return ap.rearrange(
        "n b (c4 t x) d f g k l -> n t (b c4) (x d f g k l)", c4=4, t=TPN)


def emit_body(nc, cpool, wpool, ppool, opool, poses_d, w_d, msel_d, v_ap):
    f16, f32 = mybir.dt.float16, mybir.dt.float32

    # ---- loads (Activation HWDGE queue; stores below use the SP queue)
    poses_sb = cpool.tile([NPART, N * 4 * HH], f16, tag="poses")
    nc.scalar.dma_start(out=poses_sb[:], in_=poses_d.ap())
    w_sb = cpool.tile([NPART, 8 * D * KK], f16, tag="w")
    nc.scalar.dma_start(out=w_sb[:], in_=w_d.ap())
    msel_sb = cpool.tile([NPART, NPART], f16, tag="msel")
    nc.scalar.dma_start(out=msel_sb[:], in_=msel_d.ap())

    # ---- p-sum tree within each row: (n, 4, hh) -> (n, hh)
    tmp1 = wpool.tile([NPART, N * 2 * HH], f16, tag="tmp1")
    poses_v = poses_sb[:].rearrange("q (n p hh) -> q n p hh", n=N, p=4)
    tmp1_v = tmp1[:].rearrange("q (n p hh) -> q n p hh", n=N, p=2)
    nc.vector.tensor_add(out=tmp1_v, in0=poses_v[:, :, 0:2, :],
                         in1=poses_v[:, :, 2:4, :])
    partial = wpool.tile([NPART, N * HH], f16, tag="partial")
    partial_v = partial[:].rearrange("q (n hh) -> q n hh", n=N)
    nc.vector.tensor_add(out=partial_v, in0=tmp1_v[:, :, 0, :],
                         in1=tmp1_v[:, :, 1, :])

    # ---- cross-partition group-of-4 reduce, replicated to the group (PE)
    sfull = ppool.tile([NPART, N * HH], f32, tag="sfull")
    nc.tensor.matmul(out=sfull[:], lhsT=msel_sb[:], rhs=partial[:],
                     start=True, stop=True)

    # ---- unfold s(14,14) -> s(f,g,k,l), cast to f16
    sdup = wpool.tile([NPART, N * FF * KK], f16, tag="sdup")
    sfull_v = sfull[:].rearrange("q (n i j) -> q n i j", n=N, i=H)
    sdup_v = sdup[:].rearrange("q (n f g k l) -> q n f g k l",
                               n=N, f=F, g=F, k=K)
    for k in range(K):
        for l in range(K):
            nc.vector.tensor_copy(
                out=sdup_v[:, :, :, :, k, l],
                in_=sfull_v[:, :, k:k + 2 * F - 1:2, l:l + 2 * F - 1:2])

    # ---- multiply + store per (n, channel-quad)
    sdup_nv = sdup[:].rearrange("q (n fg kl) -> q n fg kl", n=N, kl=KK)
    for n in range(N):
        for t in range(TPN):
            out_t = opool.tile([NPART, TILE_FREE], f16, tag="out")
            out_v = out_t[:].rearrange("q (xd fg kl) -> q xd fg kl",
                                       xd=XPT * D, fg=FF)
            w_view = w_sb[:, t * XPT * D * KK:(t + 1) * XPT * D * KK] \
                .rearrange("q (xd kl) -> q xd kl", kl=KK)[:, :, None, :] \
                .broadcast_to((NPART, XPT * D, FF, KK))
            s_view = sdup_nv[:, n][:, None, :, :] \
                .broadcast_to((NPART, XPT * D, FF, KK))
            nc.vector.tensor_mul(out=out_v, in0=w_view, in1=s_view)
            nc.sync.dma_start(out=v_ap[n, t], in_=out_t[:])


def build_nc(reps=1, with_scratch=False):
    """Build+compile the NEFF. reps>1 unrolls the body for steady-state
    timing; odd reps write to a DRAM scratch so every rep does identical
    DMA volume without aliasing the graded output."""
    f16 = mybir.dt.float16
    nc = bacc.Bacc("TRN2", target_bir_lowering=False)
    poses_d = nc.dram_tensor("poses", (NPART, N * 4 * HH), f16,
                             kind="ExternalInput")
    w_d = nc.dram_tensor("W", (NPART, 8 * D * KK), f16, kind="ExternalInput")
    msel_d = nc.dram_tensor("Msel", (NPART, NPART), f16, kind="ExternalInput")
    V = nc.dram_tensor("V", (N, B, C, D, F, F, K, K), f16,
                       kind="ExternalOutput")
    with TileContext(nc) as tc:
        with tc.tile_pool(name="const", bufs=2) as cpool, \
             tc.tile_pool(name="work", bufs=2) as wpool, \
             tc.tile_pool(name="psum", bufs=2, space="PSUM") as ppool, \
             tc.tile_pool(name="out", bufs=4) as opool:
            if with_scratch:
                with tc.tile_pool(name="dram", bufs=1, space="DRAM") as dpool:
                    vscr = dpool.tile([N, B, C, D, F, F, K, K], f16)
                    for r in range(reps):
                        tgt = _v_ap(V.ap()) if r % 2 == 0 else _v_ap(vscr[:])
                        emit_body(nc, cpool, wpool, ppool, opool,
                                  poses_d, w_d, msel_d, tgt)
            else:
                for r in range(reps):
                    emit_body(nc, cpool, wpool, ppool, opool,
                              poses_d, w_d, msel_d, _v_ap(V.ap()))
    nc.compile()
    return nc


# ---- host-side prep (not part of device time) ----

def prep_poses(shard: np.ndarray) -> np.ndarray:
    """(N,B,P,H,H) f32 core shard -> (128, N*4*HH) f16, row q=b*4+c4 holds
    poses[:, b, 4*c4:4*c4+4] as (n, p4, hh)."""
    t = shard.reshape(N, B, 4, 4, HH).transpose(1, 2, 0, 3, 4)
    return np.ascontiguousarray(t.reshape(NPART, N * 4 * HH)).astype(np.float16)


def prep_W(Wf: np.ndarray) -> np.ndarray:
    """(B,C,D,K,K) -> (128, 8*D*KK) f16: row q=b*4+c4 = W[b, 8*c4:8*c4+8]."""
    return np.ascontiguousarray(Wf.reshape(NPART, 8 * D * KK)).astype(np.float16)


def make_msel() -> np.ndarray:
    """Block-diagonal group-of-4 selection matrix, f16."""
    return np.ascontiguousarray(
        np.kron(np.eye(32, dtype=np.float16), np.ones((4, 4), np.float16)))


_cached_nc = None


def _get_nc():
    global _cached_nc
    if _cached_nc is None:
        _cached_nc = build_nc(reps=1)
    return _cached_nc


def make_in_maps(poses: np.ndarray, W: np.ndarray):
    Wp = prep_W(W)
    msel = make_msel()
    return [{"poses": prep_poses(poses[i * N:(i + 1) * N]),
             "W": Wp, "Msel": msel} for i in range(NCORES)]


def run_spmd(poses: np.ndarray, W: np.ndarray, **spmd_kwargs):
    """Shard, run on 8 cores, gather. Returns (V_full f32, results)."""
    poses = np.ascontiguousarray(np.asarray(poses, dtype=np.float32))
    W = np.ascontiguousarray(np.asarray(W, dtype=np.float32))
    assert poses.shape == (NTOT, B, P, H, H), poses.shape
    assert W.shape == (B, C, D, K, K), W.shape
    nc = _get_nc()
    res = bass_utils.run_bass_kernel_spmd(nc, make_in_maps(poses, W),
                                          core_ids=list(range(NCORES)),
                                          **spmd_kwargs)
    V = np.concatenate([r["V"] for r in res.results], axis=0).astype(np.float32)
    return V, res


def kernel(poses: np.ndarray, W: np.ndarray) -> np.ndarray:
    import time
    last_err = None
    for attempt in range(3):
        try:
            V, _ = run_spmd(poses, W)
            return V
        except Exception as e:  # transient NRT/axon device errors: poke + retry
            last_err = e
            time.sleep(2.0)
            try:
                import jax, jax.numpy as jnp
                jnp.sum(jnp.ones((8, 8))).block_until_ready()
            except Exception:
                pass
    raise last_err
